# revision 30
# baseline (speedup 1.0000x reference)
"""AngularPenaltySMLoss (CosFace) on 8 TRN2 NeuronCores.

Strategy: tensor-parallel over the class dimension C=100000. Each core owns
12500 classes (zero-padded to 12800 = 25 c-tiles of 512). Per core:
  - logits tile [128 n x 512 c] = bf16 matmul of xT (stationary) against
    W-shard^T (moving), K=512 contracted in 4 accumulating PSUM steps;
    c-tiles processed in chunks of 4 (one 4-bank PSUM group per chunk)
  - one wide ScalarE Exp activation per PSUM group with per-partition scale
    a[n] = S/||x_n|| (folds the L2 normalization and the S factor into the
    activation) and the fused row-sum accumulator -> per-sample partial
    exp-sums. Each zero pad class contributes exactly exp(0)=1; the total
    padding count is subtracted as a constant in the epilogue.
  - target logits from host-gathered W[labels] rows (data movement only):
    DVE multiply + row-reduce per n-tile (deferred past the main loop).
  - AllReduce (16 KB) of the per-sample partial sums, then the CosFace
    log/denominator epilogue on-device; every core emits the same scalar.

xT and the W shard are pre-cast to bf16 on the host (pure dtype/layout
prep) to halve their DMA volume; x/W[labels] stay f32 for the norm/target
paths.
"""

import ml_dtypes
import numpy as np

from concourse import bacc, mybir, tile
from concourse.bass_utils import run_bass_kernel_spmd

N, D, C = 4096, 512, 100000
N_CORES = 8
C_SHARD = C // N_CORES          # 12500
CT = 512                        # class-tile width (one PSUM bank of f32)
NJ = 25                         # c-tiles per core -> C_PAD = 12800
S = 30.0
SM = 10.5                       # S * margin(0.35)

# Schraudolph fast-exp constants (DVE offload): exp(x) ~= bitcast_f32(
# int32(x * 2^23/ln2 + (127*2^23 - C))), C=486411 zeroes the mean error
EXP_A = float(2 ** 23 / np.log(2))
EXP_B = float(1065353216 - 486411)

f32 = mybir.dt.float32
bf16 = mybir.dt.bfloat16
fp8 = mybir.dt.float8e4
i32 = mybir.dt.int32
np_bf16 = ml_dtypes.bfloat16
np_fp8 = mybir.dt.np(mybir.dt.float8e4)
AF = mybir.ActivationFunctionType
ALU = mybir.AluOpType
AX = mybir.AxisListType


def build(n=N, d=D, c_shard=C_SHARD, ct=CT, nj=NJ, n_cores=N_CORES,
          use_fp8=True):
    ni = n // 128
    nk = d // 128
    c_pad = nj * ct
    pad_corr = float(n_cores * (c_pad - c_shard))
    if use_fp8:
        assert d % 256 == 0
        nk8 = d // 256        # DoubleRow contracts 256 per step
    mm_dt = fp8 if use_fp8 else bf16

    # group c-tiles into chunks of up to 4 (one PSUM group, one wide ACT)
    chunks = []
    off = 0
    while off < nj:
        sz = min(4, nj - off)
        chunks.append((off, sz))
        off += sz
    n_chunks = len(chunks)
    psg_w = max(sz for _, sz in chunks) * ct

    nc = bacc.Bacc("TRN2", target_bir_lowering=False, debug=False,
                   num_devices=n_cores)
    x_nat = nc.dram_tensor("x_nat", [n, d], bf16, kind="ExternalInput").ap()
    xtb_d = nc.dram_tensor("xtb", [d, n], mm_dt, kind="ExternalInput").ap()
    wl = nc.dram_tensor("wl", [n, d], bf16, kind="ExternalInput").ap()
    wt = nc.dram_tensor("wt", [d, c_pad], mm_dt, kind="ExternalInput").ap()
    out = nc.dram_tensor("out", [1, 1], f32, kind="ExternalOutput").ap()

    with tile.TileContext(nc) as tc:
        with (
            tc.tile_pool(name="persist", bufs=1) as pp,
            tc.tile_pool(name="stage", bufs=3) as sp,
            tc.tile_pool(name="wbuf", bufs=2 * 4 * nk) as wbp,
            tc.tile_pool(name="scr", bufs=2) as scp,
            tc.tile_pool(name="dram", bufs=1, space="DRAM") as dp,
        ):
            if use_fp8:
                xtb = [pp.tile([128, 2, n], fp8, tag=f"xtb{g}",
                               name=f"xtbs{g}") for g in range(nk8)]
            else:
                xtb = [pp.tile([128, n], bf16, tag=f"xtb{k}", name=f"xtbs{k}")
                       for k in range(nk)]
            parts = pp.tile([128, ni * n_chunks], f32, tag="parts",
                            name="parts")
            ss = pp.tile([128, ni], f32, tag="ss", name="ss")
            tgt = pp.tile([128, ni], f32, tag="tgt", name="tgt")
            u = pp.tile([128, ni], f32, tag="u", name="u")
            a_all = pp.tile([128, ni], f32, tag="a_all", name="a_all")
            a2_all = pp.tile([128, ni], f32, tag="a2_all", name="a2_all")
            tot = pp.tile([128, ni], f32, tag="tot", name="tot")
            loc = pp.tile([128, ni], f32, tag="loc", name="loc")
            ones = pp.tile([128, 1], f32, tag="ones", name="ones")

            # xT resident in SBUF (fp8/bf16 straight from HBM) -- gates the
            # first matmuls, so issue these DMAs first
            if use_fp8:
                for g in range(nk8):
                    nc.sync.dma_start(
                        xtb[g][:],
                        xtb_d[g * 256:(g + 1) * 256, :].rearrange(
                            "(s p) n -> p s n", s=2))
            else:
                for k in range(nk):
                    nc.sync.dma_start(xtb[k][:],
                                      xtb_d[k * 128:(k + 1) * 128, :])

            # norms: the Exp activations need a_all; compute in batches of 8
            # n-tiles so a_all columns become available incrementally
            # a[n] = S / ||x_n|| = 1 / sqrt(ss / S^2)
            for b0 in range(0, ni, 8):
                b1 = min(b0 + 8, ni)
                for i in range(b0, b1):
                    xa = sp.tile([128, d], bf16, tag="xa", name="xa")
                    nc.sync.dma_start(xa[:], x_nat[i * 128:(i + 1) * 128, :])
                    sq = scp.tile([128, d], f32, tag="sq", name="sq")
                    nc.scalar.activation(sq[:], xa[:], AF.Square,
                                         accum_out=ss[:, i:i + 1])
                nc.scalar.activation(u[:, b0:b1], ss[:, b0:b1], AF.Sqrt,
                                     scale=1.0 / (S * S))
                nc.vector.reciprocal(a_all[:, b0:b1], u[:, b0:b1])
                nc.vector.tensor_scalar_mul(a2_all[:, b0:b1],
                                            a_all[:, b0:b1], EXP_A)
            nc.vector.memset(ones[:], 1.0)

            # main loop: chunks of up to 4 c-tiles x ni n-tiles.
            # One PSUM group ([128, sz*ct], up to 4 banks) per (i, chunk);
            # a single wide Exp activation with fused row-sum accumulator
            # consumes the whole group.
            # target-logit work for n-tile i (DVE mul + row reduce); spread
            # across main-loop chunks so it never forms a serial tail
            def tgt_work(i):
                xa2 = sp.tile([128, d], bf16, tag="xa2", name="xa2")
                nc.sync.dma_start(xa2[:], x_nat[i * 128:(i + 1) * 128, :])
                wla = sp.tile([128, d], bf16, tag="wla", name="wla")
                nc.sync.dma_start(wla[:], wl[i * 128:(i + 1) * 128, :])
                pr = scp.tile([128, d], f32, tag="pr", name="pr")
                nc.vector.tensor_mul(pr[:], xa2[:], wla[:])
                nc.vector.reduce_sum(tgt[:, i:i + 1], pr[:], axis=AX.X)

            tgt_done = set()
            with tc.tile_pool(name="psum", bufs=2, space="PSUM") as psp:
                for ci, (j0, sz) in enumerate(chunks):
                    wbt = {}
                    if use_fp8:
                        for jc in range(sz):
                            for g in range(nk8):
                                j = j0 + jc
                                wb = wbp.tile([128, 2, ct], fp8, tag="wb",
                                              name="wb")
                                nc.sync.dma_start(
                                    wb[:],
                                    wt[g * 256:(g + 1) * 256,
                                       j * ct:(j + 1) * ct].rearrange(
                                           "(s p) c -> p s c", s=2))
                                wbt[(jc, g)] = wb
                    else:
                        for jc in range(sz):
                            for k in range(nk):
                                j = j0 + jc
                                wb = wbp.tile([128, ct], bf16, tag="wb",
                                              name="wb")
                                nc.sync.dma_start(
                                    wb[:], wt[k * 128:(k + 1) * 128,
                                              j * ct:(j + 1) * ct])
                                wbt[(jc, k)] = wb
                    for i in range(ni):
                        ps = psp.tile([128, psg_w], f32, tag="ps", name="ps")
                        if use_fp8:
                            for g in range(nk8):
                                lhs = xtb[g][:, :, i * 128:(i + 1) * 128]
                                for jc in range(sz):
                                    nc.tensor.matmul(
                                        ps[:, jc * ct:(jc + 1) * ct], lhs,
                                        wbt[(jc, g)][:],
                                        start=(g == 0), stop=(g == nk8 - 1),
                                        perf_mode=(
                                            mybir.MatmulPerfMode.DoubleRow))
                        else:
                            for k in range(nk):
                                lhs = xtb[k][:, i * 128:(i + 1) * 128]
                                for jc in range(sz):
                                    nc.tensor.matmul(
                                        ps[:, jc * ct:(jc + 1) * ct], lhs,
                                        wbt[(jc, k)][:],
                                        start=(k == 0), stop=(k == nk - 1))
                        col = i * n_chunks + ci
                        if sz == 4 and (i + ci) % 4 == 0:
                            # DVE fast-exp offload (Schraudolph bit trick):
                            # frees ScalarE, which is otherwise the
                            # bottleneck. Never applied to the padded tail
                            # chunk (pads must contribute exactly 1.0).
                            tf = scp.tile([128, psg_w], f32, tag="tf",
                                          name="tf")
                            nc.vector.tensor_scalar(
                                out=tf[:], in0=ps[:],
                                scalar1=a2_all[:, i:i + 1], scalar2=EXP_B,
                                op0=ALU.mult, op1=ALU.add)
                            ti = scp.tile([128, psg_w], i32, tag="ti",
                                          name="ti")
                            nc.vector.tensor_copy(ti[:], tf[:])
                            nc.vector.reduce_sum(parts[:, col:col + 1],
                                                 ti[:].bitcast(f32),
                                                 axis=AX.X)
                        else:
                            es = scp.tile([128, psg_w], bf16, tag="es",
                                          name="es")
                            nc.scalar.activation(
                                es[:, :sz * ct], ps[:, :sz * ct], AF.Exp,
                                scale=a_all[:, i:i + 1],
                                accum_out=parts[:, col:col + 1])
                        # interleave tgt work (chunks 1..4, 8 tiles each)
                        if 1 <= ci and i % 4 == 3:
                            it = (ci - 1) * 8 + i // 4
                            if it < ni and it not in tgt_done:
                                tgt_work(it)
                                tgt_done.add(it)

            # any tgt tiles not covered by the interleave (small configs)
            for i in range(ni):
                if i not in tgt_done:
                    tgt_work(i)

            # per-sample local sum over this core's chunks
            for i in range(ni):
                nc.vector.reduce_sum(
                    loc[:, i:i + 1],
                    parts[:, i * n_chunks:(i + 1) * n_chunks], axis=AX.X)

            cc_in = dp.tile([128, ni], f32, name="cc_in")
            cc_out = dp.tile([128, ni], f32, addr_space="Shared", name="cc_out")
            nc.sync.dma_start(cc_in[:], loc[:])
            nc.gpsimd.collective_compute(
                "AllReduce", ALU.add,
                replica_groups=[list(range(n_cores))],
                ins=[cc_in[:]], outs=[cc_out[:]])
            nc.sync.dma_start(tot[:], cc_out[:])

            # epilogue: loss = mean(log(den) - S*tgt) + S*margin
            t1 = pp.tile([128, ni], f32, tag="t1", name="t1")
            e1 = pp.tile([128, ni], f32, tag="e1", name="e1")
            e2 = pp.tile([128, ni], f32, tag="e2", name="e2")
            den = pp.tile([128, ni], f32, tag="den", name="den")
            lg = pp.tile([128, ni], f32, tag="lg", name="lg")
            v = pp.tile([128, ni], f32, tag="v", name="v")
            rowv = pp.tile([128, 1], f32, tag="rowv", name="rowv")
            res = pp.tile([1, 1], f32, tag="res", name="res")

            nc.vector.tensor_mul(t1[:], a_all[:], tgt[:])     # S * tgt cosine
            nc.scalar.activation(e2[:], t1[:], AF.Exp)
            # exp(t1 - SM) == exp(t1) * exp(-SM); const bias APs for -SM
            # aren't registered, so fold via DVE scalar-mul instead
            nc.vector.tensor_scalar_mul(e1[:], e2[:], float(np.exp(-SM)))
            # (tot - pad_corr) - e2
            nc.vector.scalar_tensor_tensor(out=den[:], in0=tot[:],
                                           scalar=-pad_corr, in1=e2[:],
                                           op0=ALU.add, op1=ALU.subtract)
            nc.vector.tensor_add(den[:], den[:], e1[:])
            nc.scalar.activation(lg[:], den[:], AF.Ln)
            nc.vector.tensor_sub(v[:], lg[:], t1[:])
            nc.vector.reduce_sum(rowv[:], v[:], axis=AX.X)
            with tc.tile_pool(name="psum1", bufs=1, space="PSUM") as psp1:
                pss = psp1.tile([1, 1], f32, tag="pss", name="pss")
                nc.tensor.matmul(pss[:], rowv[:], ones[:], start=True,
                                 stop=True)
                nc.vector.tensor_scalar_mul(res[:], pss[:], 1.0 / n)
                nc.vector.tensor_scalar_add(res[:], res[:], SM)
            nc.sync.dma_start(out[:], res[:])

    nc.compile()
    return nc


def in_maps(x, W, labels, c_shard=C_SHARD, ct=CT, nj=NJ, n_cores=N_CORES,
            use_fp8=True):
    d = x.shape[1]
    c_pad = nj * ct
    np_mm = np_fp8 if use_fp8 else np_bf16
    x = np.ascontiguousarray(np.asarray(x, dtype=np.float32))
    W = np.ascontiguousarray(np.asarray(W, dtype=np.float32))
    lab = np.asarray(labels).astype(np.int64)
    xb = np.ascontiguousarray(x.astype(np_bf16))
    xtb = np.ascontiguousarray(x.T.astype(np_mm))
    wlg = np.ascontiguousarray(W[lab].astype(np_bf16))
    maps = []
    for c in range(n_cores):
        wt = np.zeros((d, c_pad), np_mm)
        wt[:, :c_shard] = W[c * c_shard:(c + 1) * c_shard].T.astype(np_mm)
        maps.append({"x_nat": xb, "xtb": xtb, "wl": wlg, "wt": wt})
    return maps


_CACHE = {}


def _get_nc():
    if "nc" not in _CACHE:
        _CACHE["nc"] = build()
    return _CACHE["nc"]


def kernel(x, W, labels):
    nc = _get_nc()
    res = run_bass_kernel_spmd(nc, in_maps(x, W, labels),
                               core_ids=list(range(N_CORES)))
    val = np.asarray(res.results[0]["out"], dtype=np.float32)
    return val.reshape(())
